# revision 5
# baseline (speedup 1.0000x reference)
"""GroupedQueryAttention (block-local attention) Trainium2 kernel.

Problem: x[4, 4096, 1024] -> fused QKV (w_qkv [3072,1024]) -> 16 heads,
dh=64 -> attention within 8 sequence blocks of 512 -> out proj
(w_out [1024,1024] + b_out).

Sharding: one sequence block (512 tokens x 4 batches) per NeuronCore;
attention is block-local so there are no cross-core collectives.

Device layout strategy (all matmuls fp32r at full PE rate, N=512):
  - host pre-transposes x and weights so no on-device transposes needed
  - Q^T,K^T computed in [d, n] layout (bf16 storage) = scores lhsT/rhs
  - V computed in natural [n, d] layout with a ones column appended ->
    the attention@V matmul also yields softmax denominators for free
  - attention output accumulates transposed [d, n] = final proj lhsT
  - softmax skips max-subtraction (scores ~ N(0,1), exp is safe in fp32)
  - bias is added via a K=1 matmul appended to the proj accumulation
"""
import numpy as np

import concourse.bass as bass
import concourse.mybir as mybir
from concourse.tile import TileContext
from concourse.bass_utils import run_bass_kernel_spmd

FP32 = mybir.dt.float32
FP32R = mybir.dt.float32r
BF16 = mybir.dt.bfloat16
AF = mybir.ActivationFunctionType

B, L, D = 4, 4096, 1024
H, DH = 16, 64
G, LG = 8, 512  # blocks, block length (one block per core)
NCORES = 8
N = B * LG  # tokens per core = 2048

_WAIT_CAP = 1


def _split_excess_waits(nc):
    """walrus caps sync-wait slots per instruction; move overflow waits
    onto injected same-engine drains placed just before the instruction."""
    ctr = 0
    for fn in nc.m.functions:
        for blk in fn.blocks:
            insts = blk.instructions
            new_list = []
            for inst in insts:
                si = inst.sync_info
                if si is not None and len(si.on_wait) > _WAIT_CAP:
                    waits = list(si.on_wait)
                    keep, overflow = waits[:_WAIT_CAP], waits[_WAIT_CAP:]
                    for j in range(0, len(overflow), _WAIT_CAP):
                        d = mybir.InstDrain(name=f"I-ws-{ctr}", ins=[], outs=[])
                        ctr += 1
                        d.engine = inst.engine
                        d.sync_info = mybir.SyncInfo(
                            on_wait=overflow[j : j + _WAIT_CAP], on_update=[]
                        )
                        nc.register_instruction(d, overwrite=True)
                        new_list.append(d)
                    inst.sync_info = mybir.SyncInfo(
                        on_wait=keep, on_update=si.on_update
                    )
                new_list.append(inst)
            insts[:] = new_list
    return ctr


def build():
    nc = bass.Bass(dynamic_dma_scratch_size=512)
    xt_d = nc.declare_dram_parameter("xt", [D, N], FP32R, isOutput=False)
    wq_d = nc.declare_dram_parameter("wq", [D, 3 * D], FP32R, isOutput=False)
    wo_d = nc.declare_dram_parameter("wo", [D, D], FP32R, isOutput=False)
    br_d = nc.declare_dram_parameter("br", [1, D], FP32R, isOutput=False)
    oc_d = nc.declare_dram_parameter("onescol", [128, 64], FP32R, isOutput=False)
    on_d = nc.declare_dram_parameter("ones", [1, 128], FP32R, isOutput=False)
    out_d = nc.declare_dram_parameter("out", [N, D], FP32, isOutput=True)

    xt_ap = xt_d[:].rearrange("(c p) n -> p c n", p=128)  # [128, 8, 2048]
    wq_ap = wq_d[:].rearrange("(c p) o -> p c o", p=128)  # [128, 8, 3072]
    wo_ap = wo_d[:].rearrange("(c p) o -> p c o", p=128)  # [128, 8, 1024]
    out_ap = out_d[:].rearrange("(b c p) o -> b c p o", b=B, p=128)

    with TileContext(nc) as tc:
        with (
            nc.allow_low_precision(reason="fp32r/bf16 matmul operands"),
            tc.tile_pool(name="res", bufs=1) as res,
            tc.tile_pool(name="xtp", bufs=1) as xtp,
            tc.tile_pool(name="ep", bufs=3) as ep,
            tc.tile_pool(name="rp", bufs=2) as rp,
            tc.tile_pool(name="tbp", bufs=2) as tbp,
            tc.tile_pool(name="otp", bufs=1) as otp,
            tc.tile_pool(name="ps_q", bufs=2, space="PSUM") as ps_q,
            tc.tile_pool(name="ps_s", bufs=2, space="PSUM") as ps_s,
            tc.tile_pool(name="ps_a", bufs=2, space="PSUM") as ps_a,
            tc.tile_pool(name="ps_b", bufs=2, space="PSUM") as ps_b,
        ):
            # resident inputs
            twq = res.tile([128, 8, 3 * D], FP32R, tag="wq")
            two = res.tile([128, 8, D], FP32R, tag="wo")
            tbr = res.tile([1, D], FP32R, tag="br")
            ton = res.tile([1, 128], FP32R, tag="on")
            nc.sync.dma_start(out=twq[:], in_=wq_ap)
            nc.sync.dma_start(out=two[:], in_=wo_ap)
            nc.sync.dma_start(out=tbr[:], in_=br_d[:])
            nc.sync.dma_start(out=ton[:], in_=on_d[:])

            # per-core persistent intermediates (overwritten per batch)
            qt = res.tile([128, 8, LG], BF16, tag="qt")   # Q^T [d, l]
            kt = res.tile([128, 8, LG], BF16, tag="kt")   # K^T [d, m]
            vt = res.tile([128, 4, H, DH + 1], FP32R, tag="vt")  # V + ones col
            ao = res.tile([128, 8, LG], FP32R, tag="ao")  # AO^T [d, n]

            # ones columns of V (written once, V evictions never touch them)
            nc.sync.dma_start(
                out=vt[:, :, :, DH],
                in_=oc_d[:].rearrange("p (a b) -> p a b", a=4),
            )

            for b in range(B):
                nb = slice(b * LG, (b + 1) * LG)
                txt = xtp.tile([128, 8, LG], FP32R, tag="xt")
                nc.sync.dma_start(out=txt[:], in_=xt_ap[:, :, nb])

                # ---- QKV projections ----
                for tgt, base in ((qt, 0), (kt, D)):
                    for oc in range(8):
                        ps = ps_q.tile([128, LG], FP32, tag="pq")
                        cs = slice(base + oc * 128, base + (oc + 1) * 128)
                        for dc in range(8):
                            nc.tensor.matmul(
                                ps[:], twq[:, dc, cs], txt[:, dc, :],
                                start=(dc == 0), stop=(dc == 7),
                            )
                        nc.scalar.copy(tgt[:, oc, :], ps[:])
                for c4 in range(4):
                    for oh in range(2):
                        ps = ps_q.tile([128, LG], FP32, tag="pq")
                        cs = slice(2 * D + oh * 512, 2 * D + (oh + 1) * 512)
                        ns = slice(c4 * 128, (c4 + 1) * 128)
                        for dc in range(8):
                            nc.tensor.matmul(
                                ps[:], txt[:, dc, ns], twq[:, dc, cs],
                                start=(dc == 0), stop=(dc == 7),
                            )
                        nc.scalar.copy(
                            vt[:, c4, oh * 8 : (oh + 1) * 8, 0:DH],
                            ps[:].rearrange("p (a b) -> p a b", a=8),
                        )

                # ---- block attention, one head at a time ----
                for h in range(H):
                    hq, hp = h // 2, (h % 2) * 64
                    ets = []
                    for mcp in range(2):
                        et2 = ep.tile([128, 2, LG], FP32R, tag="e")
                        for mh in range(2):
                            mc = mcp * 2 + mh
                            sps = ps_s.tile([128, LG], FP32, tag="s")
                            nc.tensor.matmul(
                                sps[:],
                                kt[hp : hp + 64, hq, mc * 128 : (mc + 1) * 128],
                                qt[hp : hp + 64, hq, :],
                                start=True, stop=True,
                            )
                            nc.scalar.activation(
                                et2[:, mh, :], sps[:],
                                AF.Exp, scale=float(DH) ** -0.5,
                            )
                            ets.append(et2[:, mh, :])
                    aps = ps_a.tile([DH + 1, LG], FP32, tag="a")
                    for mc in range(4):
                        nc.tensor.matmul(
                            aps[:], vt[:, mc, h, :], ets[mc][:],
                            start=(mc == 0), stop=(mc == 3),
                        )
                    rt = rp.tile([1, LG], FP32R, tag="r")
                    nc.vector.reciprocal(rt[:], aps[DH : DH + 1, :])
                    bps = ps_b.tile([64, LG], FP32, tag="b")
                    nc.tensor.matmul(
                        bps[:], ton[0:1, 0:64], rt[:], start=True, stop=True
                    )
                    tb = tbp.tile([64, LG], FP32, tag="tb")
                    nc.scalar.copy(tb[:], bps[:])
                    nc.vector.tensor_tensor(
                        out=ao[hp : hp + 64, hq, :],
                        in0=aps[0:DH, :], in1=tb[:],
                        op=mybir.AluOpType.mult,
                    )

                # ---- output projection (+bias via K=1 matmul) ----
                for c4 in range(4):
                    ot = otp.tile([128, D], FP32, tag="ot")
                    ns = slice(c4 * 128, (c4 + 1) * 128)
                    for oh in range(2):
                        ps = ps_q.tile([128, 512], FP32, tag="pq")
                        os_ = slice(oh * 512, (oh + 1) * 512)
                        for dc in range(8):
                            nc.tensor.matmul(
                                ps[:], ao[:, dc, ns], two[:, dc, os_],
                                start=(dc == 0), stop=False,
                            )
                        nc.tensor.matmul(
                            ps[:], ton[0:1, 0:128], tbr[0:1, os_],
                            start=False, stop=True,
                        )
                        nc.vector.tensor_scalar_mul(ot[:, os_], ps[:], 1.0)
                    nc.sync.dma_start(out=out_ap[b, c4], in_=ot[:])

    _split_excess_waits(nc)
    return nc


_CACHED = {}


def _get_nc():
    if "nc" not in _CACHED:
        _CACHED["nc"] = build()
    return _CACHED["nc"]


def kernel(x, w_qkv, w_out, b_out):
    x = np.asarray(x, dtype=np.float32)
    w_qkv = np.asarray(w_qkv, dtype=np.float32)
    w_out = np.asarray(w_out, dtype=np.float32)
    b_out = np.asarray(b_out, dtype=np.float32)

    wq = np.ascontiguousarray(w_qkv.T)          # [1024, 3072]
    wo = np.ascontiguousarray(w_out.T)          # [1024, 1024]
    br = np.ascontiguousarray(b_out[None, :])   # [1, 1024]
    onescol = np.ones((128, 64), np.float32)
    ones = np.ones((1, 128), np.float32)

    in_maps = []
    for g in range(NCORES):
        xs = x[:, g * LG : (g + 1) * LG, :].reshape(N, D)  # [2048, 1024]
        xt = np.ascontiguousarray(xs.T)                     # [1024, 2048]
        in_maps.append({
            "xt": xt, "wq": wq, "wo": wo, "br": br,
            "onescol": onescol, "ones": ones,
        })

    nc = _get_nc()
    res = run_bass_kernel_spmd(nc, in_maps, list(range(NCORES)))

    out = np.empty((B, L, D), dtype=np.float32)
    for g in range(NCORES):
        og = res.results[g]["out"].reshape(B, LG, D)
        out[:, g * LG : (g + 1) * LG, :] = og
    return out
